# revision 58
# baseline (speedup 1.0000x reference)
"""Trainium2 Bass kernel for gated multi-head attention (nn_Attention_71751723647784).

Reference computation (B=1, Q=K=2048, CQ=CK=CV=128, H=8, CH=32, HD=256):
    q = (q_x @ Wq)/sqrt(CH); k = kv_x @ Wk; v = kv_x @ Wv           (per-head CH=32)
    a = softmax(q k^T + bias + distance.transpose(0,3,1,2), axis=-1)
    o = (a @ v) * sigmoid(q_x @ Wg + bg);  out = o @ Wo + bo

Sharding: rows of Q across the 8 cores (256 query rows per core); every input
byte read exactly once (bias is shared across heads).

v2 design (k-major scores):
  - Bulk inputs are cast/laid out in bf16 on the host: distance arrives as
    [H, 128p, 16kt*256q] (k = kt*128+p), bias as [128p, 16kt*256q], q_x/kv_x
    pre-transposed to [c, q]/[c, k], Wq pre-scaled by 1/sqrt(CH).  Halves DMA
    bytes and removes every on-chip cast / PE input transpose of v1.
  - Scores are built k-major (k on partitions): sc[k,q] = qk (PE matmul)
    + (bias+dist) where bias+dist is one DVE bf16 add (2x mode) merged into
    PSUM with one PE identity-matmul.  exp on ACT reads PSUM directly; no
    max-subtraction needed (scores are O(6), bf16/f32 range is ample).
  - Softmax normalisation is deferred past the AV matmul: v carries an
    appended ones-column, so row 32 of each head's AV output is the softmax
    denominator for free.  recip via reciprocal_approx_fast, broadcast over
    the 32 ch rows with a tiny PE outer-product, folded into the gating mults.
  - No e-transposes (33us of DMA_TRANSPOSE in v1) and no separate
    normalisation pass over e (12us of DVE in v1).
"""

import math
import numpy as np

B, Q, KS = 1, 2048, 2048
CQ = 128
H, CH = 8, 32
HD = H * CH  # 256
NCORES = 8
QL = Q // NCORES       # 256 query rows per core
NKT = 16               # k tiles of 128
KW = NKT * QL          # 4096 score elements per partition per head
SCALE = 1.0 / math.sqrt(CH)
# exp chunks in k-tiles: 6+6+4 tiles -> ACT FD 1536/1536/1024
CHUNKS = [(0, 6), (6, 6), (12, 4)]


_CACHE = {}


def build_nc():
    from concourse import bacc
    import concourse.tile as tile
    import concourse.mybir as mybir
    f32 = mybir.dt.float32
    bf16 = mybir.dt.bfloat16
    AF = mybir.ActivationFunctionType

    nc = bacc.Bacc("TRN2", target_bir_lowering=False, debug=False)

    dist = nc.dram_tensor("distance", (H, 128, KW), bf16, kind="ExternalInput").ap()
    biasT = nc.dram_tensor("bias", (128, KW), bf16, kind="ExternalInput").ap()
    qxT = nc.dram_tensor("q_x", (CQ, QL), bf16, kind="ExternalInput").ap()
    kvxT = nc.dram_tensor("kv_x", (CQ, KS), bf16, kind="ExternalInput").ap()
    Wq = nc.dram_tensor("Wq", (CQ, HD), bf16, kind="ExternalInput").ap()  # pre-scaled
    Wk = nc.dram_tensor("Wk", (CQ, HD), bf16, kind="ExternalInput").ap()
    Wv = nc.dram_tensor("Wv", (CQ, HD), bf16, kind="ExternalInput").ap()
    Wg = nc.dram_tensor("Wg", (CQ, HD), bf16, kind="ExternalInput").ap()
    bgr = nc.dram_tensor("bg", (32, H), f32, kind="ExternalInput").ap()
    Wo = nc.dram_tensor("Wo", (32, H, CQ), bf16, kind="ExternalInput").ap()
    bo = nc.dram_tensor("bo", (1, CQ), bf16, kind="ExternalInput").ap()
    out = nc.dram_tensor("out", (QL, CQ), f32, kind="ExternalOutput").ap()

    with tile.TileContext(nc) as tc:
        with (
            tc.tile_pool(name="const", bufs=1) as constp,
            tc.tile_pool(name="wts", bufs=1) as wtp,
            tc.tile_pool(name="proj", bufs=1) as projp,
            tc.tile_pool(name="dist", bufs=3) as distp,
            tc.tile_pool(name="bd", bufs=2) as bdp,
            tc.tile_pool(name="e", bufs=3) as ep,
            tc.tile_pool(name="small", bufs=3) as smp,
            tc.tile_pool(name="psSC", bufs=2, space="PSUM") as psSC,
            tc.tile_pool(name="psX", bufs=2, space="PSUM") as psX,
        ):
            # ---- constants ----
            ones_row = constp.tile([1, QL], bf16)
            nc.gpsimd.memset(ones_row[:], 1.0)
            ones33 = constp.tile([33, 32], bf16)
            nc.gpsimd.memset(ones33[:], 1.0)

            # ---- weight / input loads (scalar HWDGE queue) ----
            wq_sb = wtp.tile([128, HD], bf16)
            wk_sb = wtp.tile([128, HD], bf16)
            wv_sb = wtp.tile([128, HD], bf16)
            wg_sb = wtp.tile([128, HD], bf16)
            wo_sb = wtp.tile([32, H, 128], bf16)
            bo_sb = wtp.tile([1, 128], bf16)
            bg_sb = wtp.tile([32, H], f32)
            qxT_sb = wtp.tile([128, QL], bf16)
            kvxT_sb = wtp.tile([128, KS], bf16)
            biasT_sb = wtp.tile([128, KW], bf16)
            nc.scalar.dma_start(qxT_sb[:], qxT)
            nc.scalar.dma_start(wg_sb[:], Wg)
            nc.scalar.dma_start(bg_sb[:], bgr)
            nc.scalar.dma_start(wk_sb[:], Wk)
            nc.scalar.dma_start(wq_sb[:], Wq)
            # bias + kvxT split so the first bd-add and the kT projection
            # can start as early as possible
            nc.scalar.dma_start(biasT_sb[:, 0:KW // 2], biasT[:, 0:KW // 2])
            for c in range(4):
                nc.scalar.dma_start(kvxT_sb[:, c * 512:(c + 1) * 512],
                                    kvxT[:, c * 512:(c + 1) * 512])
            nc.scalar.dma_start(biasT_sb[:, KW // 2:KW], biasT[:, KW // 2:KW])
            nc.scalar.dma_start(wv_sb[:], Wv)
            nc.scalar.dma_start(wo_sb[:], Wo)
            nc.scalar.dma_start(bo_sb[:], bo)

            # ---- dist prefetch: first head + the GPSIMD-assisted heads.
            # GPSIMD (otherwise idle) computes bias+dist for 3 heads, but at
            # ~9us per head it must run far ahead of consumption: issue its
            # dist loads + adds first, and process those heads late.
            # GPSIMD bd-assist disabled: its tensor_add contends for the
            # DVE's SBUF port and slows concurrent DVE ops ~4x (measured
            # 9.1us for a bd-add that normally takes 2.3us)
            GP_HEADS = (0, 4)
            HEAD_ORDER = [1, 3, 5, 0, 2, 6, 7, 4]
            d_first = distp.tile([128, KW], bf16, tag="dist")
            nc.sync.dma_start(d_first[:, 0:KW // 2], dist[HEAD_ORDER[0], :, 0:KW // 2])
            nc.sync.dma_start(d_first[:, KW // 2:KW], dist[HEAD_ORDER[0], :, KW // 2:KW])
            pre_bd = {}
            for hg in GP_HEADS:
                dg = distp.tile([128, KW], bf16, tag=f"distg{hg}", bufs=1)
                nc.sync.dma_start(dg[:], dist[hg])
                bg_ = bdp.tile([128, KW], bf16, tag=f"bdg{hg}", bufs=1)
                nc.gpsimd.tensor_add(bg_[:], dg[:], biasT_sb[:])
                pre_bd[hg] = bg_

            # ---- gating projection (its exps also warm the ACT exp table;
            # everything on ACT uses the exp set -> no table thrashing):
            #   gT8 = 1 / (1 + exp(-(Wg^T qxT + bg)))
            gT8 = projp.tile([32, H, QL], f32)
            psg1 = psSC.tile([128, 1536], f32, tag="sc", name="psg1")
            psg2 = psX.tile([128, 512], f32, tag="px", name="psg2")
            gu = smp.tile([32, H, QL], f32, tag="gu", bufs=1)
            for h in range(H):
                dst = psg1[0:32, h * QL:(h + 1) * QL] if h < 6 else \
                      psg2[0:32, (h - 6) * QL:(h - 5) * QL]
                nc.tensor.matmul(dst, lhsT=wg_sb[:, 32 * h:32 * h + 32],
                                 rhs=qxT_sb[:], start=True, stop=True)
                nc.scalar.activation(gu[:, h, :], dst, AF.Exp,
                                     scale=-1.0, bias=bg_sb[:, h:h + 1])

            # ---- bd for the first head: FIRST in the DVE queue (engines run
            # in program order; anything emitted before this would stall the
            # whole pipeline on late weight DMAs)
            bd1 = bdp.tile([128, KW], bf16, tag="bd", name="bd1")
            nc.vector.tensor_add(bd1[:, 0:KW // 2], d_first[:, 0:KW // 2],
                                 biasT_sb[:, 0:KW // 2])
            nc.vector.tensor_add(bd1[:, KW // 2:KW], d_first[:, KW // 2:KW],
                                 biasT_sb[:, KW // 2:KW])
            # finish the gating sigmoid on DVE (inputs ready ~t12);
            # ping-pong gu -> gT8 -> gu to avoid in-place DVE ops (slow)
            nc.vector.tensor_scalar_add(
                gT8[:].rearrange("c a q -> c (a q)"),
                gu[:].rearrange("c a q -> c (a q)"), 1.0)
            nc.vector.reciprocal_approx_fast(
                out=gu[:].rearrange("c a q -> c (a q)"),
                in_=gT8[:].rearrange("c a q -> c (a q)"))
            gT8 = gu  # the final gate lives in gu

            # qT[g][hd-half, q] (Wq pre-scaled on host)
            qT = []
            for g in range(2):
                psq = psX.tile([128, 512], f32, tag="px", name=f"psq{g}")
                nc.tensor.matmul(psq[:, 0:QL], lhsT=wq_sb[:, g * 128:(g + 1) * 128],
                                 rhs=qxT_sb[:], start=True, stop=True)
                qt = projp.tile([128, QL], bf16, tag=f"qT{g}", name=f"qT{g}")
                nc.vector.tensor_copy(qt[:], psq[:, 0:QL])
                qT.append(qt)
            # kT[g][hd-half, k] full width
            kT = []
            for g in range(2):
                kt_ = projp.tile([128, KS], bf16, tag=f"kT{g}", name=f"kT{g}")
                for c in range(2):
                    psk = psSC.tile([128, 1536], f32, tag="sc", name=f"psk{g}{c}")
                    for j in range(2):
                        nc.tensor.matmul(
                            psk[:, j * 512:(j + 1) * 512],
                            lhsT=wk_sb[:, g * 128:(g + 1) * 128],
                            rhs=kvxT_sb[:, c * 1024 + j * 512:c * 1024 + (j + 1) * 512],
                            start=True, stop=True)
                    nc.vector.tensor_copy(kt_[:, c * 1024:(c + 1) * 1024],
                                          psk[:, 0:1024])
                kT.append(kt_)

            v1 = projp.tile([128, NKT, H, 36], bf16)
            go_all = projp.tile([128, H, QL], bf16)  # rows 0-31 live

            def emit_scores(h, bd, split_e2=False):
                """e = exp(qk) * exp(bias+dist): keeps the PE out of the
                bias/dist merge (the PE clock throttles under load, so PE
                cycles are the scarcest resource). exp is exact; the product
                is a DVE bf16 2x op."""
                g, hl = h // 4, h % 4
                e_sb = ep.tile([128, KW], bf16, tag="e", name=f"e{h}")
                e2 = bdp.tile([128, KW], bf16, tag="e2", name=f"e2_{h}")
                if split_e2:
                    nc.scalar.activation(e2[:, 0:KW // 2], bd[:, 0:KW // 2],
                                         AF.Exp)
                    nc.scalar.activation(e2[:, KW // 2:KW], bd[:, KW // 2:KW],
                                         AF.Exp)
                else:
                    nc.scalar.activation(e2[:], bd[:], AF.Exp)
                for ci, (kt0, nkt) in enumerate(CHUNKS):
                    sc = psSC.tile([128, 1536], f32, tag="sc")
                    for j in range(nkt):
                        kt = kt0 + j
                        nc.tensor.matmul(
                            sc[:, j * QL:(j + 1) * QL],
                            lhsT=kT[g][32 * hl:32 * hl + 32,
                                       kt * 128:(kt + 1) * 128],
                            rhs=qT[g][32 * hl:32 * hl + 32, :],
                            start=True, stop=True,
                            tile_position=(32 * hl, 0))
                    e1 = smp.tile([128, 1536], bf16, tag="e1")
                    nc.scalar.activation(e1[:, 0:nkt * QL],
                                         sc[:, 0:nkt * QL], AF.Exp)
                    nc.vector.tensor_mul(
                        e_sb[:, kt0 * QL:(kt0 + nkt) * QL],
                        e1[:, 0:nkt * QL],
                        e2[:, kt0 * QL:(kt0 + nkt) * QL])
                return e_sb

            def emit_av_gate(h, e_sb):
                # AV with ones-column: rows 0-31 data, row 32 = denominator
                av = psX.tile([128, 512], f32, tag="px", name=f"av{h}")
                avs = av[0:33, 0:QL]
                for kt in range(NKT):
                    nc.tensor.matmul(avs, lhsT=v1[:, kt, h, 0:33],
                                     rhs=e_sb[:, kt * QL:(kt + 1) * QL],
                                     start=(kt == 0), stop=(kt == NKT - 1))
                # denominator -> recip -> broadcast over 32 rows -> gating
                dn = smp.tile([33, QL], bf16, tag="dn")
                nc.vector.tensor_copy(dn[32:33, :], av[32:33, 0:QL])
                px = psX.tile([128, 512], f32, tag="px", name=f"denB{h}")
                nc.tensor.matmul(px[0:32, 0:QL], lhsT=ones33[32:33, :],
                                 rhs=dn[32:33, :], start=True, stop=True)
                recipB = smp.tile([32, QL], f32, tag="recipB")
                nc.vector.reciprocal_approx_fast(out=recipB[:],
                                                 in_=px[0:32, 0:QL])
                t1 = smp.tile([32, QL], bf16, tag="t1")
                nc.vector.tensor_mul(t1[:], av[0:32, 0:QL], gT8[:, h, :])
                nc.vector.tensor_mul(go_all[0:32, h, :], t1[:], recipB[:])

            # ---- first head's scores, then the v projection (its weight DMA
            # lands late; emitting it earlier would block the DVE/PE queues),
            # then the remaining heads
            e_first = emit_scores(HEAD_ORDER[0], bd1, split_e2=True)

            nc.gpsimd.memset(v1[:, :, :, 32:33], 1.0)
            for kt2 in range(NKT // 2):
                psv = psX.tile([128, 512], f32, tag="px", name=f"psv{kt2}")
                for j in range(2):
                    nc.tensor.matmul(psv[:, j * HD:(j + 1) * HD],
                                     lhsT=kvxT_sb[:, (2 * kt2 + j) * 128:
                                                  (2 * kt2 + j + 1) * 128],
                                     rhs=wv_sb[:], start=True, stop=True)
                nc.vector.tensor_copy(
                    v1[:, 2 * kt2:2 * kt2 + 2, :, 0:32],
                    psv[:, 0:2 * HD].rearrange("p (a h c) -> p a h c", a=2, h=H))

            emit_av_gate(HEAD_ORDER[0], e_first)

            for h in HEAD_ORDER[1:]:
                if h in GP_HEADS:
                    bd = pre_bd[h]
                else:
                    d_sb = distp.tile([128, KW], bf16, tag="dist")
                    nc.sync.dma_start(d_sb[:], dist[h])
                    bd = bdp.tile([128, KW], bf16, tag="bd")
                    nc.vector.tensor_add(bd[:], d_sb[:], biasT_sb[:])
                e_sb = emit_scores(h, bd)
                emit_av_gate(h, e_sb)

            # ---- output projection ----
            for qt in range(2):
                qsl = slice(qt * 128, (qt + 1) * 128)
                pso = psX.tile([128, 512], f32, tag="px", name=f"pso{qt}")
                for h in range(H):
                    nc.tensor.matmul(pso[:, 0:128], lhsT=go_all[0:32, h, qsl],
                                     rhs=wo_sb[:, h, :], start=(h == 0), stop=False)
                nc.tensor.matmul(pso[:, 0:128], lhsT=ones_row[:, 0:128],
                                 rhs=bo_sb[:], start=False, stop=True)
                out_sb = smp.tile([128, 128], f32, tag="out")
                nc.vector.tensor_copy(out_sb[:], pso[:, 0:128])
                nc.sync.dma_start(
                    out.rearrange("(a p) c -> a p c", p=128)[qt], out_sb[:])

    nc.compile()
    return nc


def _get_nc():
    if "nc" not in _CACHE:
        _CACHE["nc"] = build_nc()
    return _CACHE["nc"]


def make_in_maps(q_x, kv_x, bias, distance, Wq, Wk, Wv, Wg, bg, Wo, bo):
    import ml_dtypes
    bf = ml_dtypes.bfloat16
    com = {
        "kv_x": np.ascontiguousarray(np.asarray(kv_x[0]).T).astype(bf),
        "Wq": (np.asarray(Wq) * SCALE).astype(bf),
        "Wk": np.asarray(Wk).astype(bf),
        "Wv": np.asarray(Wv).astype(bf),
        "Wg": np.asarray(Wg).astype(bf),
        # negated: consumed as exp(-x + bias) in the sigmoid-via-exp path
        "bg": np.ascontiguousarray(
            -np.asarray(bg, np.float32).reshape(H, 32).T),
        "Wo": np.ascontiguousarray(
            np.asarray(Wo).reshape(H, 32, CQ).transpose(1, 0, 2)).astype(bf),
        "bo": np.asarray(bo).reshape(1, CQ).astype(bf),
    }
    maps = []
    for i in range(NCORES):
        s = slice(i * QL, (i + 1) * QL)
        m = dict(com)
        m["q_x"] = np.ascontiguousarray(np.asarray(q_x[0, s]).T).astype(bf)
        # bias[q,k] -> [p, kt*q] with k = kt*128 + p
        bslc = np.asarray(bias[0, 0, s])                              # [q, k]
        m["bias"] = np.ascontiguousarray(
            bslc.T.reshape(NKT, 128, QL).transpose(1, 0, 2).reshape(128, KW)
        ).astype(bf)
        # distance[q,k,h] -> [h, p, kt*q]
        dslc = np.asarray(distance[0, s])                             # [q, k, h]
        m["distance"] = np.ascontiguousarray(
            dslc.transpose(2, 1, 0).reshape(H, NKT, 128, QL)
                .transpose(0, 2, 1, 3).reshape(H, 128, KW)).astype(bf)
        maps.append(m)
    return maps


def kernel(q_x, kv_x, bias, distance, Wq, Wk, Wv, Wg, bg, Wo, bo, trace=False):
    from concourse.bass_utils import run_bass_kernel_spmd

    nc = _get_nc()
    in_maps = make_in_maps(q_x, kv_x, bias, distance, Wq, Wk, Wv, Wg, bg, Wo, bo)
    res = run_bass_kernel_spmd(nc, in_maps, core_ids=list(range(NCORES)),
                               trace=trace)
    _CACHE["last_result"] = res
    out = np.concatenate([res.results[i]["out"] for i in range(NCORES)], axis=0)
    return out.reshape(B, Q, CQ).astype(np.float32)
